# revision 20
# baseline (speedup 1.0000x reference)
"""Trainium2 Bass kernel for nn_ActionSpikeEncode (spiking CNN encoder).

Sharding: data-parallel over batch B=1024 across 8 NeuronCores (128 each).
BatchNorm batch statistics are made global via an on-device AllReduce of
per-core partial sums (2*1024 floats per BN layer).

Precision strategy: the reference output is a mean of binary spikes, so the
output is exact unless a spike flips.  All matmuls therefore run at
fp32-class accuracy: conv1 in plain fp32; conv2 and the FC layer as a
two-pass fp32r (11-bit round-to-nearest mantissa) hi/lo weight split, which
is equivalent to ~23-bit weights since the activations are exactly
representable binary spikes.  The LIF / BN elementwise chains mirror the
reference op-for-op (same rounding order) on the vector engine.

Layout: [h on partitions, h-chunk, l, b] everywhere, so the innermost free
dim is b=128 (even), satisfying the fp32r access-pattern restrictions.
"""

import numpy as np
from contextlib import ExitStack

T, B, CIN, L, H = 8, 1024, 16, 8, 1024
NCORES = 8
BS = B // NCORES        # 128 batch per core
HC = H // 128           # 8 chunks of 128 channels
O = 1024                # fc output channels (CIN * 64)
L2 = L // 2             # 4 (after pool1)
L4 = L2 // 2            # 2 (after pool2)
NF = 16                 # fc contraction chunks (2048 / 128)
EPS = 1e-5
STAGE_A = (0, 1, 2)     # conv2 output-channel chunk groups
STAGE_B = (3, 4, 5)
STAGE_C = (6, 7)

TRACE = False           # set True (by test harness) to capture HW exec time
LAST = {}               # exec_time_ns / trace path stash for the harness

_CACHE = {}


def _rne11(x):
    """Round f32 to 11 explicit mantissa bits, ties-to-even (== device f32r)."""
    b = np.ascontiguousarray(x, np.float32).view(np.uint32).astype(np.uint64)
    r = (b + np.uint64(0x7FF) + ((b >> np.uint64(12)) & np.uint64(1))) & ~np.uint64(0xFFF)
    return (r & np.uint64(0xFFFFFFFF)).astype(np.uint32).view(np.float32).reshape(x.shape)


def _sigmoid32(x):
    return np.float32(1.0 / (1.0 + np.exp(-np.float64(np.float32(x)))))


def _enable_ldw_opt():
    """Rewrite the hardcoded --enable-ldw-opt=false walrus flag (big LDWEIGHTS
    overlap win for self-loading fp32r matmuls). Process-local patch."""
    import concourse.bass_utils as bu
    if getattr(bu, "_ldwopt_patched", False):
        return
    orig = bu.run_command

    def patched(argv, **kw):
        argv = [a.replace("--enable-ldw-opt=false", "--enable-ldw-opt=true")
                if isinstance(a, str) else a for a in argv]
        return orig(argv, **kw)

    bu.run_command = patched
    bu._ldwopt_patched = True


def _build(sw1, sw2, sw3, with_b1, with_b2, with_fcb, no_cc=False, phase=4, sub=9):
    import concourse.bacc as bacc
    import concourse.bass as bass
    import concourse.mybir as mybir
    import concourse.tile as tile

    f32 = mybir.dt.float32
    f32r = mybir.dt.float32r
    Alu = mybir.AluOpType
    Act = mybir.ActivationFunctionType

    nc = bacc.Bacc("TRN2", target_bir_lowering=False, debug=False,
                   enable_asserts=False, num_devices=NCORES)

    # ---- external inputs (same weight arrays on every core) ----
    xp_d = nc.dram_tensor("xp", [48, L * BS], f32, kind="ExternalInput")
    w1_d = nc.dram_tensor("w1p", [48, H], f32, kind="ExternalInput")
    w2h_d = nc.dram_tensor("w2h", [128, HC, 3, H], f32r, kind="ExternalInput")
    w2l_d = nc.dram_tensor("w2l", [128, HC, 3, H], f32r, kind="ExternalInput")
    fch_d = nc.dram_tensor("fch", [128, NF, O], f32r, kind="ExternalInput")
    fcl_d = nc.dram_tensor("fcl", [128, NF, O], f32r, kind="ExternalInput")
    bn1_d = nc.dram_tensor("bn1", [128, 3 * HC], f32, kind="ExternalInput")
    bn2_d = nc.dram_tensor("bn2", [128, 3 * HC], f32, kind="ExternalInput")
    fcb_d = nc.dram_tensor("fcb", [1, O], f32, kind="ExternalInput")

    out_d = nc.dram_tensor("out", [128, O], f32, kind="ExternalOutput")

    # ---- internal DRAM ----
    sp_d = nc.dram_tensor("sp_stash", [T, 128, HC * L2 * BS], f32r)
    y2_d = nc.dram_tensor("y2_stash", [T, HC, 128, L2 * BS], f32)
    cc1i = nc.dram_tensor("cc1i", [128, 2 * HC], f32)
    cc1o = nc.dram_tensor("cc1o", [128, 2 * HC], f32, addr_space="Shared")
    cc2i = nc.dram_tensor("cc2i", [128, 2 * HC], f32)
    cc2o = nc.dram_tensor("cc2o", [128, 2 * HC], f32, addr_space="Shared")

    with tile.TileContext(nc) as tc, ExitStack() as ctx:
        persist = ctx.enter_context(tc.tile_pool(name="persist", bufs=1))
        psum = ctx.enter_context(tc.tile_pool(name="psum", bufs=4, space="PSUM"))

        # persistent small tiles
        bn1_t = persist.tile([128, 3 * HC], f32)
        nc.sync.dma_start(out=bn1_t, in_=bn1_d[:, :])
        bn2_t = persist.tile([128, 3 * HC], f32)
        nc.sync.dma_start(out=bn2_t, in_=bn2_d[:, :])
        eps_t = persist.tile([128, 1], f32)
        nc.vector.memset(eps_t, EPS)
        st1 = persist.tile([128, 2 * HC], f32)      # BN1 partials [S | Q]
        nc.vector.memset(st1, 0.0)
        st1r = persist.tile([128, 2 * HC], f32)
        mu1 = persist.tile([128, HC], f32)
        r1 = persist.tile([128, HC], f32)
        st2c = persist.tile([128, HC * T], f32)     # BN2 sum partials (h2c, t)
        nc.vector.memset(st2c, 0.0)
        st2q = persist.tile([128, HC * T], f32)
        nc.vector.memset(st2q, 0.0)
        st2 = persist.tile([128, 2 * HC], f32)
        st2r = persist.tile([128, 2 * HC], f32)
        mu2 = persist.tile([128, HC], f32)
        r2 = persist.tile([128, HC], f32)
        s3sum = persist.tile([128, O], f32)
        nc.vector.memset(s3sum, 0.0)
        scr = persist.tile([128, 512], f32)         # ACT-accum scratch

        def bn_consts(st_r, mu, r, inv_n):
            # mu = S/N ; var = Q/N - mu^2 ; r = 1/sqrt(var + eps)
            nc.vector.tensor_scalar(out=mu, in0=st_r[:, 0:HC], scalar1=inv_n,
                                    scalar2=None, op0=Alu.mult)
            msq = persist.tile([128, HC], f32, tag="bnc_msq")
            nc.vector.tensor_scalar(out=msq, in0=st_r[:, HC:2 * HC], scalar1=inv_n,
                                    scalar2=None, op0=Alu.mult)
            mu_sq = persist.tile([128, HC], f32, tag="bnc_musq")
            nc.vector.tensor_tensor(out=mu_sq, in0=mu, in1=mu, op=Alu.mult)
            nc.vector.tensor_tensor(out=msq, in0=msq, in1=mu_sq, op=Alu.subtract)
            nc.scalar.activation(out=msq, in_=msq, func=Act.Sqrt, bias=eps_t,
                                 scale=1.0)
            nc.vector.reciprocal(out=r, in_=msq)

        def bn_apply(y_hc, hc, mu, r, bn_t):
            # y = ((y - mu) * r) * g + be   (two fused ops, reference order)
            nc.vector.tensor_scalar(out=y_hc, in0=y_hc,
                                    scalar1=mu[:, hc:hc + 1], scalar2=r[:, hc:hc + 1],
                                    op0=Alu.subtract, op1=Alu.mult)
            nc.vector.tensor_scalar(out=y_hc, in0=y_hc,
                                    scalar1=bn_t[:, HC + hc:HC + hc + 1],
                                    scalar2=bn_t[:, 2 * HC + hc:2 * HC + hc + 1],
                                    op0=Alu.mult, op1=Alu.add)

        def conv2_group(ps, wh, wl, sp, h2c_rel):
            """48 accumulating matmuls: one conv2 output chunk for one t."""
            first = True
            for wt in (wh, wl):
                for hc in range(HC):
                    def w_k(k):
                        return wt[:, hc, k, h2c_rel * 128:(h2c_rel + 1) * 128]
                    nc.tensor.matmul(ps[:, :, :], w_k(1), sp[:, hc, :, :],
                                     start=first, stop=False)
                    first = False
                    nc.tensor.matmul(ps[:, 1:L2, :], w_k(0), sp[:, hc, 0:L2 - 1, :],
                                     start=False, stop=False)
                    last = wt is wl and hc == HC - 1
                    nc.tensor.matmul(ps[:, 0:L2 - 1, :], w_k(2), sp[:, hc, 1:L2, :],
                                     start=False, stop=last)

        def conv2_post(ps, t, h2c):
            # psum -> sbuf (+bias), per-channel sums of y and y^2, stash to DRAM
            y2sb = y2p.tile([128, L2 * BS], f32, tag="y2sb")
            col = st2c[:, h2c * T + t:h2c * T + t + 1]
            if with_b2:
                nc.scalar.activation(out=y2sb, in_=ps.rearrange("p l b -> p (l b)"),
                                     func=Act.Identity,
                                     bias=bn2_t[:, h2c:h2c + 1], scale=1.0,
                                     accum_out=col)
            else:
                nc.scalar.activation(out=y2sb, in_=ps.rearrange("p l b -> p (l b)"),
                                     func=Act.Copy, accum_out=col)
            nc.scalar.activation(out=scr[:, 0:L2 * BS],
                                 in_=ps.rearrange("p l b -> p (l b)"),
                                 func=Act.Square,
                                 accum_out=st2q[:, h2c * T + t:h2c * T + t + 1])
            nc.sync.dma_start(out=y2_d[t, h2c], in_=y2sb)

        # ================= stage A: conv1, BN1, plif1, conv2 A =============
        stages_ctx = ExitStack()
        spp = stages_ctx.enter_context(tc.tile_pool(name="spbuf", bufs=2))
        y2p = stages_ctx.enter_context(tc.tile_pool(name="y2buf", bufs=3))
        with tc.tile_pool(name="stageA", bufs=1) as pA:
            convin_cm = tc.tile_pool(name="convin", bufs=1)
            convin = convin_cm.__enter__()
            xp_t = convin.tile([48, L * BS], f32)
            nc.sync.dma_start(out=xp_t, in_=xp_d[:, :])
            w1_t = convin.tile([48, H], f32)
            nc.sync.dma_start(out=w1_t, in_=w1_d[:, :])
            nA = len(STAGE_A)
            w2hA = pA.tile([128, HC, 3, nA * 128], f32r)
            nc.sync.dma_start(out=w2hA, in_=w2h_d[:, :, :, 0:nA * 128])
            w2lA = pA.tile([128, HC, 3, nA * 128], f32r)
            nc.sync.dma_start(out=w2lA, in_=w2l_d[:, :, :, 0:nA * 128])

            # ---- conv1 + BN1 partial stats ----
            y1 = pA.tile([128, HC, L, BS], f32)
            nc.vector.memset(y1.rearrange("p c l b -> p (c l b)"), 0.0)
            st1h = pA.tile([128, 4 * HC], f32)   # per-half partials [S0 S1 Q0 Q1]
            for hc in range(HC if sub >= 2 else 0):
                for half in range(2):
                    ps = psum.tile([128, L2, BS], f32, tag="ps")
                    nc.tensor.matmul(
                        ps.rearrange("p l b -> p (l b)"),
                        w1_t[:, hc * 128:(hc + 1) * 128],
                        xp_t[:, half * 512:(half + 1) * 512],
                        start=True, stop=True)
                    y_half = y1[:, hc, half * L2:(half + 1) * L2, :].rearrange(
                        "p l b -> p (l b)")
                    nc.scalar.activation(
                        out=y_half, in_=ps.rearrange("p l b -> p (l b)"),
                        func=Act.Copy,
                        accum_out=st1h[:, 2 * hc + half:2 * hc + half + 1])
                    nc.scalar.activation(
                        out=scr[:, 0:512], in_=ps.rearrange("p l b -> p (l b)"),
                        func=Act.Square,
                        accum_out=st1h[:, 2 * HC + 2 * hc + half:
                                       2 * HC + 2 * hc + half + 1])
                y_hc = y1[:, hc].rearrange("p l b -> p (l b)")
                if with_b1:
                    # bias must precede stats: fall back to DVE reductions
                    nc.vector.tensor_scalar(out=y_hc, in0=y_hc,
                                            scalar1=bn1_t[:, hc:hc + 1],
                                            scalar2=None, op0=Alu.add)
                    nc.vector.tensor_reduce(out=st1[:, hc:hc + 1], in_=y_hc,
                                            axis=mybir.AxisListType.X, op=Alu.add)
                    nc.scalar.activation(out=scr, in_=y_hc[:, 0:512],
                                         func=Act.Square,
                                         accum_out=st1h[:, 2 * HC + 2 * hc:
                                                        2 * HC + 2 * hc + 1])
                    nc.scalar.activation(out=scr, in_=y_hc[:, 512:1024],
                                         func=Act.Square,
                                         accum_out=st1h[:, 2 * HC + 2 * hc + 1:
                                                        2 * HC + 2 * hc + 2])
                    nc.vector.tensor_reduce(
                        out=st1[:, HC + hc:HC + hc + 1],
                        in_=st1h[:, 2 * HC + 2 * hc:2 * HC + 2 * hc + 2],
                        axis=mybir.AxisListType.X, op=Alu.add)
            convin_cm.__exit__(None, None, None)
            if not with_b1:
                nc.vector.tensor_reduce(
                    out=st1[:, 0:HC],
                    in_=st1h[:, 0:2 * HC].rearrange("p (c h) -> p c h", c=HC),
                    axis=mybir.AxisListType.X, op=Alu.add)
                nc.vector.tensor_reduce(
                    out=st1[:, HC:2 * HC],
                    in_=st1h[:, 2 * HC:].rearrange("p (c h) -> p c h", c=HC),
                    axis=mybir.AxisListType.X, op=Alu.add)
            # ---- allreduce BN1 stats; apply BN1 in place (y1 -> u1) ----
            if no_cc:
                nc.vector.tensor_scalar(out=st1r, in0=st1, scalar1=float(NCORES),
                                        scalar2=None, op0=Alu.mult)
            else:
                nc.sync.dma_start(out=cc1i[:, :], in_=st1)
                nc.gpsimd.collective_compute(
                    "AllReduce", Alu.add, replica_groups=[list(range(NCORES))],
                    ins=[cc1i[:, :]], outs=[cc1o[:, :]])
                nc.sync.dma_start(out=st1r, in_=cc1o[:, :])
            if sub >= 4:
                bn_consts(st1r, mu1, r1, 1.0 / (B * L))
                for hc in range(HC):
                    bn_apply(y1[:, hc].rearrange("p l b -> p (l b)"), hc, mu1, r1,
                             bn1_t)

            # ---- plif1 (t-sequential) + conv2 stage-A chunks ----
            v1 = pA.tile([128, HC, L, BS], f32)
            nc.vector.memset(v1.rearrange("p c l b -> p (c l b)"), 0.0)
            u1f = y1.rearrange("p c l b -> p (c l b)")
            v1f = v1.rearrange("p c l b -> p (c l b)")
            scrA_cm = tc.tile_pool(name="scrA", bufs=1)
            scrA = scrA_cm.__enter__()
            HLF = HC * L * BS // 2
            for t in range(T if phase >= 2 else 0):
                sp = spp.tile([128, HC, L2, BS], f32r, tag="sp")
                for h in range(2):
                    d = scrA.tile([128, HLF], f32, tag="dA")
                    uh = u1f[:, h * HLF:(h + 1) * HLF]
                    vh = v1f[:, h * HLF:(h + 1) * HLF]
                    nc.vector.tensor_tensor(out=d, in0=uh, in1=vh, op=Alu.subtract)
                    nc.vector.scalar_tensor_tensor(out=vh, in0=d, scalar=sw1,
                                                   in1=vh, op0=Alu.mult, op1=Alu.add)
                vp = scrA.tile([128, HC, L2, BS], f32, tag="dA")
                nc.vector.tensor_tensor(out=vp, in0=v1[:, :, 0:L:2, :],
                                        in1=v1[:, :, 1:L:2, :], op=Alu.max)
                nc.vector.tensor_scalar(out=sp, in0=vp, scalar1=1.0,
                                        scalar2=None, op0=Alu.is_ge)
                nc.vector.scalar_tensor_tensor(out=v1f, in0=v1f, scalar=1.0,
                                               in1=v1f, op0=Alu.is_lt,
                                               op1=Alu.mult)
                nc.sync.dma_start(out=sp_d[t],
                                  in_=sp.rearrange("p c l b -> p (c l b)"))
                for h2c in STAGE_A:
                    ps = psum.tile([128, L2, BS], f32, tag="ps")
                    conv2_group(ps, w2hA, w2lA, sp, h2c)
                    conv2_post(ps, t, h2c)
            scrA_cm.__exit__(None, None, None)

        # ============ stages B/C: conv2 remaining chunks ===================
        def conv_stage(pool_, chunks, first_abs):
            wh = pool_.tile([128, HC, 3, len(chunks) * 128], f32r,
                            tag=f"w2h_{first_abs}")
            wl = pool_.tile([128, HC, 3, len(chunks) * 128], f32r,
                            tag=f"w2l_{first_abs}")
            for j in range(len(chunks)):
                lo, hi = (first_abs + j) * 128, (first_abs + j + 1) * 128
                nc.sync.dma_start(out=wh[:, :, :, j * 128:(j + 1) * 128],
                                  in_=w2h_d[:, :, :, lo:hi])
                nc.sync.dma_start(out=wl[:, :, :, j * 128:(j + 1) * 128],
                                  in_=w2l_d[:, :, :, lo:hi])
            for t in range(T if phase >= 3 else 0):
                spb = spp.tile([128, HC, L2, BS], f32r, tag="sp")
                nc.sync.dma_start(out=spb.rearrange("p c l b -> p (c l b)"),
                                  in_=sp_d[t])
                for j, h2c in enumerate(chunks):
                    ps = psum.tile([128, L2, BS], f32, tag="ps")
                    conv2_group(ps, wh, wl, spb, j)
                    conv2_post(ps, t, h2c)

        with tc.tile_pool(name="stageB", bufs=1) as pB:
            conv_stage(pB, STAGE_B, len(STAGE_A))
        fcpre = ctx.enter_context(tc.tile_pool(name="fcpre", bufs=1, side="right"))
        fch_t = fcpre.tile([128, NF, O], f32r)
        for oh in range(2):
            osl = slice(oh * 512, (oh + 1) * 512)
            nc.sync.dma_start(out=fch_t[:, :, osl], in_=fch_d[:, :, osl])
        with tc.tile_pool(name="stageC", bufs=1) as pC:
            conv_stage(pC, STAGE_C, len(STAGE_A) + len(STAGE_B))

        stages_ctx.close()

        # ---- allreduce BN2 stats ----
        nc.vector.tensor_reduce(out=st2[:, 0:HC],
                                in_=st2c.rearrange("p (c t) -> p c t", c=HC),
                                axis=mybir.AxisListType.X, op=Alu.add)
        nc.vector.tensor_reduce(out=st2[:, HC:2 * HC],
                                in_=st2q.rearrange("p (c t) -> p c t", c=HC),
                                axis=mybir.AxisListType.X, op=Alu.add)
        if no_cc:
            nc.vector.tensor_scalar(out=st2r, in0=st2, scalar1=float(NCORES),
                                    scalar2=None, op0=Alu.mult)
        else:
            nc.sync.dma_start(out=cc2i[:, :], in_=st2)
            nc.gpsimd.collective_compute(
                "AllReduce", Alu.add, replica_groups=[list(range(NCORES))],
                ins=[cc2i[:, :]], outs=[cc2o[:, :]])
            nc.sync.dma_start(out=st2r, in_=cc2o[:, :])
        bn_consts(st2r, mu2, r2, 1.0 / (T * B * L2))

        # ============ phase E: BN2 + plif2 + fc + plif3 + mean =============
        with tc.tile_pool(name="phaseE", bufs=1) as pE, \
             tc.tile_pool(name="y2in", bufs=2) as y2i, \
             tc.tile_pool(name="sp2buf", bufs=1) as sp2p:
            fcl_t = pE.tile([128, NF, O], f32r)
            for oh in range(2):
                osl = slice(oh * 512, (oh + 1) * 512)
                nc.sync.dma_start(out=fcl_t[:, :, osl], in_=fcl_d[:, :, osl])
            fcb_t = None
            if with_fcb:
                fcb_t = pE.tile([128, O], f32)
                ap = fcb_d[:, :]
                bcast = bass.AP(tensor=ap.tensor, offset=ap.offset,
                                ap=[[0, 128]] + ap.ap[1:])
                nc.sync.dma_start(out=fcb_t, in_=bcast)
            scrE_cm = tc.tile_pool(name="scrE", bufs=1)
            scrE = scrE_cm.__enter__()
            v2 = pE.tile([128, HC, L2, BS], f32)
            nc.vector.memset(v2.rearrange("p c l b -> p (c l b)"), 0.0)
            v3 = pE.tile([128, O], f32)
            nc.vector.memset(v3, 0.0)

            for t in range(T if phase >= 4 else 0):
                y2t = y2i.tile([128, HC, L2 * BS], f32, tag="y2t")
                nc.sync.dma_start(out=y2t,
                                  in_=y2_d[t].rearrange("c p n -> p c n"))
                sp2 = sp2p.tile([128, HC, L4, BS], f32r, tag="sp2")
                for hc in range(HC):
                    bn_apply(y2t[:, hc], hc, mu2, r2, bn2_t)
                y2f = y2t.rearrange("p c n -> p (c n)")
                v2f = v2.rearrange("p c l b -> p (c l b)")
                HLE = HC * L2 * BS // 2
                for h in range(2):
                    d2 = scrE.tile([128, HLE], f32, tag="dE")
                    yh = y2f[:, h * HLE:(h + 1) * HLE]
                    vh = v2f[:, h * HLE:(h + 1) * HLE]
                    nc.vector.tensor_tensor(out=d2, in0=yh, in1=vh, op=Alu.subtract)
                    nc.vector.scalar_tensor_tensor(out=vh, in0=d2, scalar=sw2,
                                                   in1=vh, op0=Alu.mult, op1=Alu.add)
                vp2 = scrE.tile([128, HC, L4, BS], f32, tag="dE")
                nc.vector.tensor_tensor(out=vp2, in0=v2[:, :, 0:L2:2, :],
                                        in1=v2[:, :, 1:L2:2, :], op=Alu.max)
                nc.vector.tensor_scalar(out=sp2, in0=vp2, scalar1=1.0,
                                        scalar2=None, op0=Alu.is_ge)
                nc.vector.scalar_tensor_tensor(out=v2f, in0=v2f, scalar=1.0,
                                               in1=v2f, op0=Alu.is_lt,
                                               op1=Alu.mult)
                for oh in range(2):
                    psf = psum.tile([128, 512], f32, tag="ps")
                    first = True
                    for wt in (fch_t, fcl_t):
                        for c in range(NF):
                            nc.tensor.matmul(
                                psf, sp2[:, c % HC, c // HC, :],
                                wt[:, c, oh * 512:(oh + 1) * 512],
                                start=first,
                                stop=(wt is fcl_t and c == NF - 1))
                            first = False
                    v3h = v3[:, oh * 512:(oh + 1) * 512]
                    if with_fcb:
                        nc.vector.tensor_tensor(
                            out=psf, in0=psf,
                            in1=fcb_t[:, oh * 512:(oh + 1) * 512], op=Alu.add)
                    d3 = scrE.tile([128, 512], f32, tag="dE")
                    nc.vector.tensor_tensor(out=d3, in0=psf, in1=v3h,
                                            op=Alu.subtract)  # psum: DVE only
                    nc.vector.scalar_tensor_tensor(out=v3h, in0=d3, scalar=sw3,
                                                   in1=v3h, op0=Alu.mult,
                                                   op1=Alu.add)
                    s3h = s3sum[:, oh * 512:(oh + 1) * 512]
                    nc.vector.scalar_tensor_tensor(out=s3h, in0=v3h, scalar=1.0,
                                                   in1=s3h, op0=Alu.is_ge,
                                                   op1=Alu.add)
                    nc.vector.scalar_tensor_tensor(out=v3h, in0=v3h, scalar=1.0,
                                                   in1=v3h, op0=Alu.is_lt,
                                                   op1=Alu.mult)

            scrE_cm.__exit__(None, None, None)
            nc.vector.tensor_scalar(out=s3sum, in0=s3sum, scalar1=1.0 / T,
                                    scalar2=None, op0=Alu.mult)
            if phase < 4:
                nc.vector.tensor_copy(out=s3sum[:, 0:2 * HC], in_=st1r)
                nc.vector.tensor_copy(out=s3sum[:, 2 * HC:4 * HC], in_=st2r)
            nc.sync.dma_start(out=out_d[:, :], in_=s3sum)

    nc.compile()
    return nc


def kernel(x, w1, b1, g1, be1, pw1, w2, b2, g2, be2, pw2, fcw, fcb, pw3):
    from concourse.bass_utils import run_bass_kernel_spmd

    x = np.asarray(x, np.float32)
    w1 = np.asarray(w1, np.float32)
    b1 = np.asarray(b1, np.float32)
    g1 = np.asarray(g1, np.float32)
    be1 = np.asarray(be1, np.float32)
    w2 = np.asarray(w2, np.float32)
    b2 = np.asarray(b2, np.float32)
    g2 = np.asarray(g2, np.float32)
    be2 = np.asarray(be2, np.float32)
    fcw = np.asarray(fcw, np.float32)
    fcb = np.asarray(fcb, np.float32)

    sw1 = float(_sigmoid32(np.asarray(pw1)))
    sw2 = float(_sigmoid32(np.asarray(pw2)))
    sw3 = float(_sigmoid32(np.asarray(pw3)))
    with_b1 = bool(np.any(b1 != 0))
    with_b2 = bool(np.any(b2 != 0))
    with_fcb = bool(np.any(fcb != 0))

    # ---- host-side input marshalling (layout only, no math) ----
    # conv1 im2col patches: patches[c, k, l, b] = x[b, c, l+k-1] (zero padded)
    xpad = np.zeros((B, CIN, L + 2), np.float32)
    xpad[:, :, 1:L + 1] = x
    patches = np.empty((CIN, 3, L, B), np.float32)
    for k in range(3):
        patches[:, k] = xpad[:, :, k:k + L].transpose(1, 2, 0)
    w1p = np.ascontiguousarray(w1.reshape(H, CIN * 3).T)       # [(c,k), h]

    # conv2 weights [p, hc, k, h2], hi/lo fp32r split (host RNE-11 == device)
    w2t = np.ascontiguousarray(
        w2.reshape(H, HC, 128, 3).transpose(2, 1, 3, 0))       # [p, hc, k, h2]
    w2h = _rne11(w2t)
    w2l = _rne11(w2t - w2h)

    # fc weights: f = 2*h + l; chunk c = l*HC + hc; fcp[p, c, o]
    fcwt = fcw.reshape(O, H, 2)
    fcp = np.empty((128, NF, O), np.float32)
    for c in range(NF):
        hc, l = c % HC, c // HC
        fcp[:, c, :] = fcwt[:, hc * 128:(hc + 1) * 128, l].T
    fch = _rne11(fcp)
    fcl = _rne11(fcp - fch)

    def chan_pack(a, b_, c_):
        return np.concatenate(
            [v.reshape(HC, 128).T for v in (a, b_, c_)], axis=1).astype(np.float32)

    bn1 = np.ascontiguousarray(chan_pack(b1, g1, be1))
    bn2 = np.ascontiguousarray(chan_pack(b2, g2, be2))

    import os
    if os.environ.get("KERNEL_LDWOPT"):
        _enable_ldw_opt()
    key = (sw1, sw2, sw3, with_b1, with_b2, with_fcb,
           bool(os.environ.get("KERNEL_NO_CC")),
           int(os.environ.get("KERNEL_PHASE", "4")),
           int(os.environ.get("KERNEL_SUB", "9")))
    if key not in _CACHE:
        _CACHE[key] = _build(*key)
    nc = _CACHE[key]

    shared = dict(w1p=w1p, w2h=w2h, w2l=w2l, fch=fch, fcl=fcl,
                  bn1=bn1, bn2=bn2, fcb=np.ascontiguousarray(fcb.reshape(1, O)))
    in_maps = []
    for c in range(NCORES):
        m = dict(shared)
        m["xp"] = np.ascontiguousarray(
            patches[:, :, :, c * BS:(c + 1) * BS].reshape(48, L * BS))
        in_maps.append(m)

    res = run_bass_kernel_spmd(nc, in_maps, core_ids=list(range(NCORES)),
                               trace=TRACE)
    LAST["exec_time_ns"] = res.exec_time_ns
    LAST["trace"] = (res.instructions_and_trace or (None, None))[1]

    out = np.concatenate([res.results[c]["out"] for c in range(NCORES)], axis=0)
    return out.reshape(B, CIN, -1)
